# revision 28
# baseline (speedup 1.0000x reference)
"""KAN layer kernel for 8 Trainium2 NeuronCores.

Math (reference):
    basis[b,f] = sum_h silu(x[b,f]*w1[f%K,h] + b1[f%K,h]) * w2[f%K,h] + b2[f%K]
    out[b,o]   = sum_f basis[b,f] * Wsum[o,f],   Wsum = W.sum(-1)   # [O,F]

Sharding: features split 8 ways (each core holds ~2048 of the 16384
features and produces a partial out[64,1024]; host sums the partials).

Per-core device program (memory-bound on streaming its ~21 MB W slice):
  - W is cast to fp16 on the host (tolerance is 2e-2; fp16 keeps us ~5e-4)
    and laid out [tile, k, partition, o] so the k-reduction folds into the
    matmul contraction: no accum-DMA chains, no on-chip k-sum. The stream
    uses 7 large HWDGE DMAs (16-32 KB per-partition lines): big descriptors
    keep the SDMA engines gap-free (many small chunks pay a per-dma_start
    ring bubble), and all W tiles are SBUF-resident so nothing stalls it.
  - Features are permuted so each SBUF partition only holds features of a
    single f%K residue class. The silu affine params are then per-partition
    constants valid across every tile, so the basis needs 32 wide ACT
    instructions + 32 wide DVE accumulates instead of 256+256 narrow ones
    (ACT costs (N+352)/1.2 ns per instr -- narrow = overhead-dominated).
  - The basis is computed in two column halves (tiles 0-7 | 8-16). W tiles
    are streamed t-major, so the PE starts contracting the first chunks
    as soon as half A is done (~22 us) while half B computes, and the run
    stays DMA-stream-bound even if compute clocks throttle.
  - Slot grid is 17 deep per partition: 16 full [128 x 64b] basis tiles plus
    one 32-partition spill tile absorbing the residue-class remainders
    (class sizes aren't multiples of 16).
  - 2 PSUM banks accumulate out[64,1024] across all 85 contraction tiles;
    the final chunk is 2 tiles so the post-stream tail is 4 matmuls + copy.
"""
import numpy as np

B, I, O, K, H = 64, 16384, 1024, 5, 16
NCORES = 8
P = 128                   # SBUF partitions
NT = 16                   # full tiles (main slot grid depth)
T = NT + 1                # slots per partition incl. spill slot
M = 20                    # spill-tile partition count (max residue spill)
TB = T * B                # basis free dim: 17*64 = 1088
# Basis tile groups (start, ntiles): the PE can begin contracting group 0
# ~8 us after the first silu instead of waiting for the whole 17-tile chain.
GROUPS = [(0, 4), (4, 4), (8, 4), (12, 5)]
# W tiles stream t-major (tau = t*K + k) so early chunks only need half A.
# 16 KB per-partition lines: big enough for a gap-free SDMA stream, small
# enough to dodge the PE clock throttle seen with 32-44 KB descriptors.
# The last 2 tiles are split by output column half (Wh1/Wh0) so ps1's
# accumulation closes early and its copy+store overlap ps0's final matmuls.
CHUNKS = [8, 8, 8, 8, 8, 8, 8, 8, 8, 6]
PR = 3 * H + 1            # per-partition silu params: w1,b1,w2 + b2 (fp32)
XC = TB + 2 * PR          # const cols: fp16 x block + fp32 params as col pairs

TRACE = False             # test.py sets True to capture an NTFF profile
LAST_RESULT = None


def _plan_core(feats_by_class):
    """Assign one core's features to the (partition, slot) grid.

    Returns (cls_of_p[P], F17[P, T]) with F17 holding feature ids, -1 = pad.
    Every partition holds features of exactly one f%K class; spill slots
    (slot NT) only on partitions < M.
    """
    ks = [len(f) for f in feats_by_class]
    n = [-(-k // T) for k in ks]              # ceil(k/17) partitions minimum
    spare = P - sum(n)
    assert spare >= 0, (ks, n)
    for _ in range(spare):                     # kill the biggest spills first
        spills = [max(0, ks[c] - NT * n[c]) for c in range(K)]
        c = int(np.argmax(spills))
        n[c] += 1
    units = []                                 # (class, main[<=16], spill|-1)
    for c in range(K):
        fs = list(feats_by_class[c])
        main, sp = fs[: NT * n[c]], fs[NT * n[c]:]
        assert len(sp) <= n[c]
        for i in range(n[c]):
            units.append((c, main[NT * i: NT * (i + 1)],
                          sp[i] if i < len(sp) else -1))
    units.sort(key=lambda u: u[2] < 0)         # spill-carrying partitions first
    assert len(units) <= P
    n_spill = sum(1 for u in units if u[2] >= 0)
    assert n_spill <= M, n_spill
    units += [(0, [], -1)] * (P - len(units))
    cls_of_p = np.array([u[0] for u in units], dtype=np.int64)
    F17 = np.full((P, T), -1, dtype=np.int64)
    for p, (c, main, sp) in enumerate(units):
        F17[p, : len(main)] = main
        F17[p, NT] = sp
    return cls_of_p, F17


def _build():
    from contextlib import ExitStack
    from concourse import bacc, mybir, tile

    f32, f16 = mybir.dt.float32, mybir.dt.float16
    mult, add = mybir.AluOpType.mult, mybir.AluOpType.add
    nc = bacc.Bacc("TRN2", target_bir_lowering=False, debug=False,
                   num_devices=NCORES)
    Wms = [nc.declare_dram_parameter(f"Wm{i}", [P, sz * O], f16, isOutput=False)
           for i, sz in enumerate(CHUNKS)]
    Wh1 = nc.declare_dram_parameter("Wh1", [P, 1024], f16, isOutput=False)
    Wh0 = nc.declare_dram_parameter("Wh0", [P, 1024], f16, isOutput=False)
    Wp = nc.declare_dram_parameter("Wp", [M, K * O], f16, isOutput=False)
    xd = nc.declare_dram_parameter("xd", [P, XC], f16, isOutput=False)
    out = nc.declare_dram_parameter("out", [B, O], f32, isOutput=True)

    with tile.TileContext(nc) as tc, ExitStack() as ctx:
        const = ctx.enter_context(tc.tile_pool(name="const", bufs=1))
        wpool = ctx.enter_context(tc.tile_pool(name="w", bufs=1))
        wsp = ctx.enter_context(tc.tile_pool(name="wsp", bufs=1))
        spool = ctx.enter_context(tc.tile_pool(name="silu", bufs=4))
        apool = ctx.enter_context(tc.tile_pool(name="acc", bufs=1))
        opool = ctx.enter_context(tc.tile_pool(name="out", bufs=1))
        psum = ctx.enter_context(tc.tile_pool(name="psum", bufs=1, space="PSUM"))

        # x+params first on the SP HWDGE queue (basis can start ~3 us after
        # the preamble), then the ~21 MB W stream owns it end to end. The
        # spill W rides the idle GpSimd SWDGE queue.
        xt = const.tile([P, XC], f16)
        nc.sync.dma_start(xt[:, :], xd[:, :])
        wtiles = []
        for c, sz in enumerate(CHUNKS):
            wt = wpool.tile([P, sz * O], f16, name=f"w{c}")
            nc.sync.dma_start(wt[:, :], Wms[c][:, :])
            wtiles.append((wt, sz))
        wh1 = wpool.tile([P, 1024], f16, name="wh1")
        nc.sync.dma_start(wh1[:, :], Wh1[:, :])
        wh0 = wpool.tile([P, 1024], f16, name="wh0")
        nc.sync.dma_start(wh0[:, :], Wh0[:, :])
        wpt = wsp.tile([M, K * O], f16)
        nc.gpsimd.dma_start(wpt[:, :], Wp[:, :])

        # ---- basis: acc[p, t*B+b] = sum_h silu(x*w1+b1)*w2 + b2, fp16,
        # in 4 column groups so the PE can start on group 0 early ----
        accs = []

        def prm(i):          # i-th fp32 param, stored as an fp16 col pair
            return xt[:, TB + 2 * i:TB + 2 * i + 2].bitcast(f32)

        for gi, (t0, nt) in enumerate(GROUPS):
            lo, w = t0 * B, nt * B
            acc = apool.tile([P, w], f16, name=f"acc{gi}")
            accs.append(acc)
            for h in range(H):
                st = spool.tile([P, w], f16, tag=f"st{gi}")
                nc.scalar.activation(
                    st[:, :], xt[:, lo:lo + w],
                    mybir.ActivationFunctionType.Silu,
                    bias=prm(H + h), scale=prm(h))
                if h == 0:
                    nc.vector.tensor_scalar(
                        acc[:, :], st[:, :], prm(2 * H), prm(3 * H),
                        op0=mult, op1=add)
                else:
                    nc.vector.scalar_tensor_tensor(
                        acc[:, :], st[:, :], prm(2 * H + h),
                        acc[:, :], op0=mult, op1=add)

        # ---- matmuls: out[b,o] += acc_tile.T @ W_tile over 85 tiles ----
        ps0 = psum.tile([B, 512], f32, tag="ps0")
        ps1 = psum.tile([B, 512], f32, tag="ps1")
        nmm = K * NT + K          # accumulation length per PSUM bank
        n0 = n1 = 0

        def mm0(lhsT, rhs):
            nonlocal n0
            nc.tensor.matmul(ps0[:, :], lhsT, rhs,
                             start=(n0 == 0), stop=(n0 == nmm - 1))
            n0 += 1

        def mm1(lhsT, rhs):
            nonlocal n1
            nc.tensor.matmul(ps1[:, :], lhsT, rhs,
                             start=(n1 == 0), stop=(n1 == nmm - 1))
            n1 += 1

        def lhs_of(t):
            gi = min(t // 4, len(GROUPS) - 1)
            lo = (t - GROUPS[gi][0]) * B
            return accs[gi][:, lo:lo + B]

        tau = 0
        for ci, (wt, sz) in enumerate(wtiles):
            if ci == 7:   # spill (needs the last group) off the critical tail
                sp = accs[-1][0:M, (NT - GROUPS[-1][0]) * B:]
                for k in range(K):
                    mm0(sp, wpt[0:M, k * O:k * O + 512])
                    mm1(sp, wpt[0:M, k * O + 512:(k + 1) * O])
            for g in range(sz):
                lhsT = lhs_of(tau // K)
                mm0(lhsT, wt[:, g * O:g * O + 512])
                mm1(lhsT, wt[:, g * O + 512:(g + 1) * O])
                tau += 1
        # final 2 tiles, ps1 columns first: ps1 closes while ps0's last
        # matmuls still stream, so its copy + store overlap them.
        for g in range(2):
            mm1(lhs_of((tau + g) // K), wh1[:, g * 512:(g + 1) * 512])
        out_sb = opool.tile([B, O], f32)
        nc.scalar.copy(out_sb[:, 512:O], ps1[:, :])
        nc.sync.dma_start(out[:, 512:O], out_sb[:, 512:O])
        for g in range(2):
            mm0(lhs_of((tau + g) // K), wh0[:, g * 512:(g + 1) * 512])
        tau += 2
        assert tau == K * NT and n0 == nmm and n1 == nmm

        nc.vector.tensor_copy(out_sb[:, 0:512], ps0[:, :])
        nc.sync.dma_start(out[:, 0:512], out_sb[:, 0:512])
    nc.compile()
    return nc


def kernel(x, w1, b1, w2, b2, W):
    global LAST_RESULT
    from concourse.bass_utils import run_bass_kernel_spmd

    x = np.asarray(x, dtype=np.float32)
    W = np.asarray(W, dtype=np.float32)
    w1 = np.asarray(w1, dtype=np.float32)
    b1 = np.asarray(b1, dtype=np.float32)
    w2 = np.asarray(w2, dtype=np.float32)
    b2 = np.asarray(b2, dtype=np.float32)

    # ---- host prep: W -> fp16 [f, k, o] with a zero row for pad slots ----
    Wt = np.zeros((I + 1, K, O), dtype=np.float16)
    Wt[:I] = W.transpose(1, 2, 0)
    xp = np.concatenate([x, np.zeros((B, 1), np.float32)], axis=1)

    in_maps = []
    seen = []
    for j in range(NCORES):
        feats = [np.arange(c, I, K)[j::NCORES] for c in range(K)]
        cls_of_p, F17 = _plan_core(feats)
        seen.append(F17[F17 >= 0].ravel())

        Fx = np.where(F17 < 0, I, F17)                 # pad -> zero col/row
        x_sb = xp[:, Fx].transpose(1, 2, 0).reshape(P, TB)
        pr = np.concatenate(
            [w1[cls_of_p], b1[cls_of_p], w2[cls_of_p], b2[cls_of_p][:, None]],
            axis=1).astype(np.float32)
        xc = np.ascontiguousarray(np.concatenate(
            [x_sb.astype(np.float16), pr.view(np.float16)], axis=1))

        A = Wt[Fx[:, :NT].T]                            # [t, p, k, o] fp16
        A = A.transpose(0, 2, 1, 3).reshape(K * NT, P, O)   # [tau=t*K+k, p, o]
        im = {"xd": xc,
              "Wp": np.ascontiguousarray(Wt[Fx[:M, NT]].reshape(M, K * O))}
        tau = 0
        for ci, sz in enumerate(CHUNKS):
            im[f"Wm{ci}"] = np.ascontiguousarray(
                A[tau:tau + sz].transpose(1, 0, 2).reshape(P, sz * O))
            tau += sz
        assert tau == K * NT - 2
        im["Wh1"] = np.ascontiguousarray(
            A[tau:, :, 512:].transpose(1, 0, 2).reshape(P, 1024))
        im["Wh0"] = np.ascontiguousarray(
            A[tau:, :, :512].transpose(1, 0, 2).reshape(P, 1024))
        in_maps.append(im)

    allf = np.sort(np.concatenate(seen))
    assert allf.shape == (I,) and np.array_equal(allf, np.arange(I))

    nc = _build()
    res = run_bass_kernel_spmd(nc, in_maps, list(range(NCORES)), trace=TRACE)
    LAST_RESULT = res
    out = np.zeros((B, O), dtype=np.float32)
    for c in range(NCORES):
        out += res.results[c]["out"]
    return out


# revision 29
# speedup vs baseline: 1.0307x; 1.0307x over previous
"""KAN layer kernel for 8 Trainium2 NeuronCores.

Math (reference):
    basis[b,f] = sum_h silu(x[b,f]*w1[f%K,h] + b1[f%K,h]) * w2[f%K,h] + b2[f%K]
    out[b,o]   = sum_f basis[b,f] * Wsum[o,f],   Wsum = W.sum(-1)   # [O,F]

Sharding: features split 8 ways (each core holds ~2048 of the 16384
features and produces a partial out[64,1024]; host sums the partials).

Per-core device program (memory-bound on streaming its ~21 MB W slice):
  - W is cast to fp16 on the host (tolerance is 2e-2; fp16 keeps us ~5e-4)
    and laid out [tile, k, partition, o] so the k-reduction folds into the
    matmul contraction: no accum-DMA chains, no on-chip k-sum. The stream
    is 10 HWDGE DMAs with 12-16 KB per-partition lines: big enough that the
    SDMA engines run gap-free (small chunks pay per-dma_start ring bubbles),
    small enough not to trip the sticky ~20% all-engine clock throttle that
    32-44 KB descriptors trigger. All W tiles are SBUF-resident (fp16 fits)
    so the stream never stalls on compute. This is the chip-level HBM
    roofline: the 8 cores' slices are disjoint, so W is read exactly once.
  - Features are permuted so each SBUF partition only holds features of a
    single f%K residue class. The silu affine params are then per-partition
    constants valid across every tile, so the basis needs 64 wide ACT
    instructions + 64 wide DVE accumulates instead of 256+256 narrow ones
    (ACT costs (N+352)/1.2 ns per instr -- narrow = overhead-dominated).
  - The basis is computed in 4 column groups (tiles 4|4|4|5) and W streams
    t-major, so the PE starts contracting chunk 0 as soon as group 0 is
    done (~21 us) and the run stays DMA-stream-bound. (No PE warm-up
    matmuls: extra concurrent activity trips the clock throttle.)
  - Slot grid is 17 deep per partition: 16 full [128 x 64b] basis tiles plus
    one 20-partition spill tile absorbing the residue-class remainders
    (class sizes aren't multiples of 16).
  - 2 PSUM banks accumulate out[64,1024] across all 85 contraction tiles.
    The last 2 W tiles are split by output column half (Wh1 then Wh0), so
    ps1 closes early and its PSUM copy + out store overlap ps0's final
    matmuls; the post-stream tail is ~4 matmuls + one copy + one store.
"""
import numpy as np

B, I, O, K, H = 64, 16384, 1024, 5, 16
NCORES = 8
P = 128                   # SBUF partitions
NT = 16                   # full tiles (main slot grid depth)
T = NT + 1                # slots per partition incl. spill slot
M = 20                    # spill-tile partition count (max residue spill)
TB = T * B                # basis free dim: 17*64 = 1088
# Basis tile groups (start, ntiles): the PE can begin contracting group 0
# ~8 us after the first silu instead of waiting for the whole 17-tile chain.
GROUPS = [(0, 4), (4, 4), (8, 4), (12, 5)]
# W tiles stream t-major (tau = t*K + k) so early chunks only need half A.
# 16 KB per-partition lines: big enough for a gap-free SDMA stream, small
# enough to dodge the PE clock throttle seen with 32-44 KB descriptors.
# The last 2 tiles are split by output column half (Wh1/Wh0) so ps1's
# accumulation closes early and its copy+store overlap ps0's final matmuls.
CHUNKS = [8, 8, 8, 8, 8, 8, 8, 8, 8, 6]
PR = 3 * H + 1            # per-partition silu params: w1,b1,w2 + b2 (fp32)
XC = TB + 2 * PR          # const cols: fp16 x block + fp32 params as col pairs

TRACE = False             # test.py sets True to capture an NTFF profile
LAST_RESULT = None


def _plan_core(feats_by_class):
    """Assign one core's features to the (partition, slot) grid.

    Returns (cls_of_p[P], F17[P, T]) with F17 holding feature ids, -1 = pad.
    Every partition holds features of exactly one f%K class; spill slots
    (slot NT) only on partitions < M.
    """
    ks = [len(f) for f in feats_by_class]
    n = [-(-k // T) for k in ks]              # ceil(k/17) partitions minimum
    spare = P - sum(n)
    assert spare >= 0, (ks, n)
    for _ in range(spare):                     # kill the biggest spills first
        spills = [max(0, ks[c] - NT * n[c]) for c in range(K)]
        c = int(np.argmax(spills))
        n[c] += 1
    units = []                                 # (class, main[<=16], spill|-1)
    for c in range(K):
        fs = list(feats_by_class[c])
        main, sp = fs[: NT * n[c]], fs[NT * n[c]:]
        assert len(sp) <= n[c]
        for i in range(n[c]):
            units.append((c, main[NT * i: NT * (i + 1)],
                          sp[i] if i < len(sp) else -1))
    units.sort(key=lambda u: u[2] < 0)         # spill-carrying partitions first
    assert len(units) <= P
    n_spill = sum(1 for u in units if u[2] >= 0)
    assert n_spill <= M, n_spill
    units += [(0, [], -1)] * (P - len(units))
    cls_of_p = np.array([u[0] for u in units], dtype=np.int64)
    F17 = np.full((P, T), -1, dtype=np.int64)
    for p, (c, main, sp) in enumerate(units):
        F17[p, : len(main)] = main
        F17[p, NT] = sp
    return cls_of_p, F17


def _build():
    from contextlib import ExitStack
    from concourse import bacc, mybir, tile

    f32, f16 = mybir.dt.float32, mybir.dt.float16
    mult, add = mybir.AluOpType.mult, mybir.AluOpType.add
    nc = bacc.Bacc("TRN2", target_bir_lowering=False, debug=False,
                   num_devices=NCORES)
    Wms = [nc.declare_dram_parameter(f"Wm{i}", [P, sz * O], f16, isOutput=False)
           for i, sz in enumerate(CHUNKS)]
    Wh1 = nc.declare_dram_parameter("Wh1", [P, 1024], f16, isOutput=False)
    Wh0 = nc.declare_dram_parameter("Wh0", [P, 1024], f16, isOutput=False)
    Wp = nc.declare_dram_parameter("Wp", [M, K * O], f16, isOutput=False)
    xd = nc.declare_dram_parameter("xd", [P, XC], f16, isOutput=False)
    out = nc.declare_dram_parameter("out", [B, O], f32, isOutput=True)

    with tile.TileContext(nc) as tc, ExitStack() as ctx:
        const = ctx.enter_context(tc.tile_pool(name="const", bufs=1))
        wpool = ctx.enter_context(tc.tile_pool(name="w", bufs=1))
        wsp = ctx.enter_context(tc.tile_pool(name="wsp", bufs=1))
        spool = ctx.enter_context(tc.tile_pool(name="silu", bufs=4))
        apool = ctx.enter_context(tc.tile_pool(name="acc", bufs=1))
        opool = ctx.enter_context(tc.tile_pool(name="out", bufs=1))
        psum = ctx.enter_context(tc.tile_pool(name="psum", bufs=1, space="PSUM"))

        # x+params first on the SP HWDGE queue (basis can start ~3 us after
        # the preamble), then the ~21 MB W stream owns it end to end. The
        # spill W rides the idle GpSimd SWDGE queue.
        xt = const.tile([P, XC], f16)
        nc.sync.dma_start(xt[:, :], xd[:, :])
        wtiles = []
        for c, sz in enumerate(CHUNKS):
            wt = wpool.tile([P, sz * O], f16, name=f"w{c}")
            nc.sync.dma_start(wt[:, :], Wms[c][:, :])
            wtiles.append((wt, sz))
        wh1 = wpool.tile([P, 1024], f16, name="wh1")
        nc.sync.dma_start(wh1[:, :], Wh1[:, :])
        wh0 = wpool.tile([P, 1024], f16, name="wh0")
        nc.sync.dma_start(wh0[:, :], Wh0[:, :])
        wpt = wsp.tile([M, K * O], f16)
        nc.gpsimd.dma_start(wpt[:, :], Wp[:, :])

        # ---- basis: acc[p, t*B+b] = sum_h silu(x*w1+b1)*w2 + b2, fp16,
        # in 4 column groups so the PE can start on group 0 early ----
        accs = []

        def prm(i):          # i-th fp32 param, stored as an fp16 col pair
            return xt[:, TB + 2 * i:TB + 2 * i + 2].bitcast(f32)

        for gi, (t0, nt) in enumerate(GROUPS):
            lo, w = t0 * B, nt * B
            acc = apool.tile([P, w], f16, name=f"acc{gi}")
            accs.append(acc)
            for h in range(H):
                st = spool.tile([P, w], f16, tag=f"st{gi}")
                nc.scalar.activation(
                    st[:, :], xt[:, lo:lo + w],
                    mybir.ActivationFunctionType.Silu,
                    bias=prm(H + h), scale=prm(h))
                if h == 0:
                    nc.vector.tensor_scalar(
                        acc[:, :], st[:, :], prm(2 * H), prm(3 * H),
                        op0=mult, op1=add)
                else:
                    nc.vector.scalar_tensor_tensor(
                        acc[:, :], st[:, :], prm(2 * H + h),
                        acc[:, :], op0=mult, op1=add)

        # ---- matmuls: out[b,o] += acc_tile.T @ W_tile over 85 tiles ----
        ps0 = psum.tile([B, 512], f32, tag="ps0")
        ps1 = psum.tile([B, 512], f32, tag="ps1")
        nmm = K * NT + K          # accumulation length per PSUM bank
        n0 = n1 = 0

        def mm0(lhsT, rhs):
            nonlocal n0
            nc.tensor.matmul(ps0[:, :], lhsT, rhs,
                             start=(n0 == 0), stop=(n0 == nmm - 1))
            n0 += 1

        def mm1(lhsT, rhs):
            nonlocal n1
            nc.tensor.matmul(ps1[:, :], lhsT, rhs,
                             start=(n1 == 0), stop=(n1 == nmm - 1))
            n1 += 1

        def lhs_of(t):
            gi = min(t // 4, len(GROUPS) - 1)
            lo = (t - GROUPS[gi][0]) * B
            return accs[gi][:, lo:lo + B]

        tau = 0
        for ci, (wt, sz) in enumerate(wtiles):
            if ci == 7:   # spill (needs the last group) off the critical tail
                sp = accs[-1][0:M, (NT - GROUPS[-1][0]) * B:]
                for k in range(K):
                    mm0(sp, wpt[0:M, k * O:k * O + 512])
                    mm1(sp, wpt[0:M, k * O + 512:(k + 1) * O])
            for g in range(sz):
                lhsT = lhs_of(tau // K)
                mm0(lhsT, wt[:, g * O:g * O + 512])
                mm1(lhsT, wt[:, g * O + 512:(g + 1) * O])
                tau += 1
        # final 2 tiles, ps1 columns first: ps1 closes while ps0's last
        # matmuls still stream, so its copy + store overlap them.
        for g in range(2):
            mm1(lhs_of((tau + g) // K), wh1[:, g * 512:(g + 1) * 512])
        out_sb = opool.tile([B, O], f32)
        nc.scalar.copy(out_sb[:, 512:O], ps1[:, :])
        nc.sync.dma_start(out[:, 512:O], out_sb[:, 512:O])
        for g in range(2):
            mm0(lhs_of((tau + g) // K), wh0[:, g * 512:(g + 1) * 512])
        tau += 2
        assert tau == K * NT and n0 == nmm and n1 == nmm

        nc.vector.tensor_copy(out_sb[:, 0:512], ps0[:, :])
        nc.sync.dma_start(out[:, 0:512], out_sb[:, 0:512])
    nc.compile()
    return nc


def kernel(x, w1, b1, w2, b2, W):
    global LAST_RESULT
    from concourse.bass_utils import run_bass_kernel_spmd

    x = np.asarray(x, dtype=np.float32)
    W = np.asarray(W, dtype=np.float32)
    w1 = np.asarray(w1, dtype=np.float32)
    b1 = np.asarray(b1, dtype=np.float32)
    w2 = np.asarray(w2, dtype=np.float32)
    b2 = np.asarray(b2, dtype=np.float32)

    # ---- host prep: W -> fp16 [f, k, o] with a zero row for pad slots ----
    Wt = np.zeros((I + 1, K, O), dtype=np.float16)
    Wt[:I] = W.transpose(1, 2, 0)
    xp = np.concatenate([x, np.zeros((B, 1), np.float32)], axis=1)

    in_maps = []
    seen = []
    for j in range(NCORES):
        feats = [np.arange(c, I, K)[j::NCORES] for c in range(K)]
        cls_of_p, F17 = _plan_core(feats)
        seen.append(F17[F17 >= 0].ravel())

        Fx = np.where(F17 < 0, I, F17)                 # pad -> zero col/row
        x_sb = xp[:, Fx].transpose(1, 2, 0).reshape(P, TB)
        pr = np.concatenate(
            [w1[cls_of_p], b1[cls_of_p], w2[cls_of_p], b2[cls_of_p][:, None]],
            axis=1).astype(np.float32)
        xc = np.ascontiguousarray(np.concatenate(
            [x_sb.astype(np.float16), pr.view(np.float16)], axis=1))

        A = Wt[Fx[:, :NT].T]                            # [t, p, k, o] fp16
        A = A.transpose(0, 2, 1, 3).reshape(K * NT, P, O)   # [tau=t*K+k, p, o]
        im = {"xd": xc,
              "Wp": np.ascontiguousarray(Wt[Fx[:M, NT]].reshape(M, K * O))}
        tau = 0
        for ci, sz in enumerate(CHUNKS):
            im[f"Wm{ci}"] = np.ascontiguousarray(
                A[tau:tau + sz].transpose(1, 0, 2).reshape(P, sz * O))
            tau += sz
        assert tau == K * NT - 2
        im["Wh1"] = np.ascontiguousarray(
            A[tau:, :, 512:].transpose(1, 0, 2).reshape(P, 1024))
        im["Wh0"] = np.ascontiguousarray(
            A[tau:, :, :512].transpose(1, 0, 2).reshape(P, 1024))
        in_maps.append(im)

    allf = np.sort(np.concatenate(seen))
    assert allf.shape == (I,) and np.array_equal(allf, np.arange(I))

    nc = _build()
    res = run_bass_kernel_spmd(nc, in_maps, list(range(NCORES)), trace=TRACE)
    LAST_RESULT = res
    out = np.zeros((B, O), dtype=np.float32)
    for c in range(NCORES):
        out += res.results[c]["out"]
    return out


# revision 32
# speedup vs baseline: 1.1088x; 1.0758x over previous
"""KAN layer kernel for 8 Trainium2 NeuronCores.

Math (reference):
    basis[b,f] = sum_h silu(x[b,f]*w1[f%K,h] + b1[f%K,h]) * w2[f%K,h] + b2[f%K]
    out[b,o]   = sum_f basis[b,f] * Wsum[o,f],   Wsum = W.sum(-1)   # [O,F]

Sharding: features split 8 ways (each core holds ~2048 of the 16384
features and produces a partial out[64,1024]; host sums the partials).

Per-core device program (memory-bound on streaming its ~21 MB W slice):
  - W is cast to fp16 on the host (tolerance is 2e-2; fp16 keeps us ~5e-4)
    and laid out [tile, k, partition, o] so the k-reduction folds into the
    matmul contraction: no accum-DMA chains, no on-chip k-sum. The stream
    is 10 HWDGE DMAs with 12-16 KB per-partition lines: big enough that the
    SDMA engines run gap-free (small chunks pay per-dma_start ring bubbles),
    small enough not to trip the sticky ~20% all-engine clock throttle that
    32-44 KB descriptors trigger. All W tiles are SBUF-resident (fp16 fits)
    so the stream never stalls on compute. This is the chip-level HBM
    roofline: the 8 cores' slices are disjoint, so W is read exactly once.
  - Features are permuted so each SBUF partition only holds features of a
    single f%K residue class. The silu affine params are then per-partition
    constants valid across every tile, so the basis needs 64 wide ACT
    instructions + 64 wide DVE accumulates instead of 256+256 narrow ones
    (ACT costs (N+352)/1.2 ns per instr -- narrow = overhead-dominated).
  - The basis is computed in 4 column groups (tiles 4|4|4|5) and W streams
    t-major, so the PE starts contracting chunk 0 as soon as group 0 is
    done (~21 us) and the run stays DMA-stream-bound. (No PE warm-up
    matmuls: extra concurrent activity trips the clock throttle.)
  - Slot grid is 17 deep per partition: 16 full [128 x 64b] basis tiles plus
    one 20-partition spill tile absorbing the residue-class remainders
    (class sizes aren't multiples of 16).
  - 2 PSUM banks accumulate out[64,1024] across all 85 contraction tiles.
    The last 2 W tiles are split by output column half (Wh1 then Wh0), so
    ps1 closes early and its PSUM copy + out store overlap ps0's final
    matmuls; the post-stream tail is ~4 matmuls + one copy + one store.
"""
import numpy as np

B, I, O, K, H = 64, 16384, 1024, 5, 16
NCORES = 8
P = 128                   # SBUF partitions
NT = 16                   # full tiles (main slot grid depth)
T = NT + 1                # slots per partition incl. spill slot
M = 20                    # spill-tile partition count (max residue spill)
TB = T * B                # basis free dim: 17*64 = 1088
# Basis tile groups (start, ntiles): the PE can begin contracting group 0
# ~8 us after the first silu instead of waiting for the whole 17-tile chain.
GROUPS = [(0, 3), (3, 4), (7, 5), (12, 5)]
# W tiles stream t-major (tau = t*K + k) so early chunks only need half A.
# 16 KB per-partition lines: big enough for a gap-free SDMA stream, small
# enough to dodge the PE clock throttle seen with 32-44 KB descriptors.
# The last 2 tiles are split by output column half (Wh1/Wh0) so ps1's
# accumulation closes early and its copy+store overlap ps0's final matmuls.
CHUNKS = [8, 8, 8, 8, 8, 8, 8, 8, 8, 6]
PR = 3 * H + 1            # per-partition silu params: w1,b1,w2 + b2 (fp32)
XC = TB + 2 * PR          # const cols: fp16 x block + fp32 params as col pairs

TRACE = False             # test.py sets True to capture an NTFF profile
LAST_RESULT = None


def _plan_core(feats_by_class):
    """Assign one core's features to the (partition, slot) grid.

    Returns (cls_of_p[P], F17[P, T]) with F17 holding feature ids, -1 = pad.
    Every partition holds features of exactly one f%K class; spill slots
    (slot NT) only on partitions < M.
    """
    ks = [len(f) for f in feats_by_class]
    n = [-(-k // T) for k in ks]              # ceil(k/17) partitions minimum
    spare = P - sum(n)
    assert spare >= 0, (ks, n)
    for _ in range(spare):                     # kill the biggest spills first
        spills = [max(0, ks[c] - NT * n[c]) for c in range(K)]
        c = int(np.argmax(spills))
        n[c] += 1
    units = []                                 # (class, main[<=16], spill|-1)
    for c in range(K):
        fs = list(feats_by_class[c])
        main, sp = fs[: NT * n[c]], fs[NT * n[c]:]
        assert len(sp) <= n[c]
        for i in range(n[c]):
            units.append((c, main[NT * i: NT * (i + 1)],
                          sp[i] if i < len(sp) else -1))
    units.sort(key=lambda u: u[2] < 0)         # spill-carrying partitions first
    assert len(units) <= P
    n_spill = sum(1 for u in units if u[2] >= 0)
    assert n_spill <= M, n_spill
    units += [(0, [], -1)] * (P - len(units))
    cls_of_p = np.array([u[0] for u in units], dtype=np.int64)
    F17 = np.full((P, T), -1, dtype=np.int64)
    for p, (c, main, sp) in enumerate(units):
        F17[p, : len(main)] = main
        F17[p, NT] = sp
    return cls_of_p, F17


def _build():
    from contextlib import ExitStack
    from concourse import bacc, mybir, tile

    f32, f16 = mybir.dt.float32, mybir.dt.float16
    mult, add = mybir.AluOpType.mult, mybir.AluOpType.add
    nc = bacc.Bacc("TRN2", target_bir_lowering=False, debug=False,
                   num_devices=NCORES)
    Wms = [nc.declare_dram_parameter(f"Wm{i}", [P, sz * O], f16, isOutput=False)
           for i, sz in enumerate(CHUNKS)]
    Wh1 = nc.declare_dram_parameter("Wh1", [P, 1024], f16, isOutput=False)
    Wh0 = nc.declare_dram_parameter("Wh0", [P, 1024], f16, isOutput=False)
    Wp = nc.declare_dram_parameter("Wp", [M, K * O], f16, isOutput=False)
    xd = nc.declare_dram_parameter("xd", [P, XC], f16, isOutput=False)
    out = nc.declare_dram_parameter("out", [B, O], f32, isOutput=True)

    with tile.TileContext(nc) as tc, ExitStack() as ctx:
        const = ctx.enter_context(tc.tile_pool(name="const", bufs=1))
        wpool = ctx.enter_context(tc.tile_pool(name="w", bufs=1))
        wsp = ctx.enter_context(tc.tile_pool(name="wsp", bufs=1))
        spool = ctx.enter_context(tc.tile_pool(name="silu", bufs=4))
        apool = ctx.enter_context(tc.tile_pool(name="acc", bufs=1))
        opool = ctx.enter_context(tc.tile_pool(name="out", bufs=1))
        psum = ctx.enter_context(tc.tile_pool(name="psum", bufs=1, space="PSUM"))

        # x+params first on the SP HWDGE queue (basis can start ~3 us after
        # the preamble), then the ~21 MB W stream owns it end to end. The
        # spill W rides the idle GpSimd SWDGE queue.
        xt = const.tile([P, XC], f16)
        nc.sync.dma_start(xt[:, :], xd[:, :])
        # Wp rides the same sync queue mid-stream (needed ~15 us after it
        # lands; avoids SWDGE entirely — its descriptor-ring traffic is the
        # suspected cause of the SDMA engine 7/15 slowdown).
        wtiles = []
        wpt = wsp.tile([M, K * O], f16)
        for c, sz in enumerate(CHUNKS):
            wt = wpool.tile([P, sz * O], f16, name=f"w{c}")
            nc.sync.dma_start(wt[:, :], Wms[c][:, :])
            wtiles.append((wt, sz))
            if c == 2:
                nc.sync.dma_start(wpt[:, :], Wp[:, :])
        wh1 = wpool.tile([P, 1024], f16, name="wh1")
        nc.sync.dma_start(wh1[:, :], Wh1[:, :])
        wh0 = wpool.tile([P, 1024], f16, name="wh0")
        nc.sync.dma_start(wh0[:, :], Wh0[:, :])

        # ---- basis: acc[p, t*B+b] = sum_h silu(x*w1+b1)*w2 + b2, fp16,
        # in 4 column groups so the PE can start on group 0 early ----
        accs = []

        def prm(i):          # i-th fp32 param, stored as an fp16 col pair
            return xt[:, TB + 2 * i:TB + 2 * i + 2].bitcast(f32)

        for gi, (t0, nt) in enumerate(GROUPS):
            lo, w = t0 * B, nt * B
            acc = apool.tile([P, w], f16, name=f"acc{gi}")
            accs.append(acc)
            for h in range(H):
                st = spool.tile([P, w], f16, tag=f"st{gi}")
                nc.scalar.activation(
                    st[:, :], xt[:, lo:lo + w],
                    mybir.ActivationFunctionType.Silu,
                    bias=prm(H + h), scale=prm(h))
                if h == 0:
                    nc.vector.tensor_scalar(
                        acc[:, :], st[:, :], prm(2 * H), prm(3 * H),
                        op0=mult, op1=add)
                else:
                    nc.vector.scalar_tensor_tensor(
                        acc[:, :], st[:, :], prm(2 * H + h),
                        acc[:, :], op0=mult, op1=add)

        # ---- matmuls: out[b,o] += acc_tile.T @ W_tile over 85 tiles ----
        ps0 = psum.tile([B, 512], f32, tag="ps0")
        ps1 = psum.tile([B, 512], f32, tag="ps1")
        nmm = K * NT + K          # accumulation length per PSUM bank
        n0 = n1 = 0

        def mm0(lhsT, rhs):
            nonlocal n0
            nc.tensor.matmul(ps0[:, :], lhsT, rhs,
                             start=(n0 == 0), stop=(n0 == nmm - 1))
            n0 += 1

        def mm1(lhsT, rhs):
            nonlocal n1
            nc.tensor.matmul(ps1[:, :], lhsT, rhs,
                             start=(n1 == 0), stop=(n1 == nmm - 1))
            n1 += 1

        def lhs_of(t):
            gi = max(i for i, (t0, _) in enumerate(GROUPS) if t >= t0)
            lo = (t - GROUPS[gi][0]) * B
            return accs[gi][:, lo:lo + B]

        tau = 0
        for ci, (wt, sz) in enumerate(wtiles):
            if ci == 7:   # spill (needs the last group) off the critical tail
                sp = accs[-1][0:M, (NT - GROUPS[-1][0]) * B:]
                for k in range(K):
                    mm0(sp, wpt[0:M, k * O:k * O + 512])
                    mm1(sp, wpt[0:M, k * O + 512:(k + 1) * O])
            for g in range(sz):
                lhsT = lhs_of(tau // K)
                mm0(lhsT, wt[:, g * O:g * O + 512])
                mm1(lhsT, wt[:, g * O + 512:(g + 1) * O])
                tau += 1
        # final 2 tiles, ps1 columns first: ps1 closes while ps0's last
        # matmuls still stream, so its copy + store overlap them.
        for g in range(2):
            mm1(lhs_of((tau + g) // K), wh1[:, g * 512:(g + 1) * 512])
        out_sb = opool.tile([B, O], f32)
        nc.scalar.copy(out_sb[:, 512:O], ps1[:, :])
        nc.sync.dma_start(out[:, 512:O], out_sb[:, 512:O])
        for g in range(2):
            mm0(lhs_of((tau + g) // K), wh0[:, g * 512:(g + 1) * 512])
        tau += 2
        assert tau == K * NT and n0 == nmm and n1 == nmm

        nc.vector.tensor_copy(out_sb[:, 0:512], ps0[:, :])
        nc.sync.dma_start(out[:, 0:512], out_sb[:, 0:512])
    nc.compile()
    return nc


def kernel(x, w1, b1, w2, b2, W):
    global LAST_RESULT
    from concourse.bass_utils import run_bass_kernel_spmd

    x = np.asarray(x, dtype=np.float32)
    W = np.asarray(W, dtype=np.float32)
    w1 = np.asarray(w1, dtype=np.float32)
    b1 = np.asarray(b1, dtype=np.float32)
    w2 = np.asarray(w2, dtype=np.float32)
    b2 = np.asarray(b2, dtype=np.float32)

    # ---- host prep: W -> fp16 [f, k, o] with a zero row for pad slots ----
    Wt = np.zeros((I + 1, K, O), dtype=np.float16)
    Wt[:I] = W.transpose(1, 2, 0)
    xp = np.concatenate([x, np.zeros((B, 1), np.float32)], axis=1)

    in_maps = []
    seen = []
    for j in range(NCORES):
        feats = [np.arange(c, I, K)[j::NCORES] for c in range(K)]
        cls_of_p, F17 = _plan_core(feats)
        seen.append(F17[F17 >= 0].ravel())

        Fx = np.where(F17 < 0, I, F17)                 # pad -> zero col/row
        x_sb = xp[:, Fx].transpose(1, 2, 0).reshape(P, TB)
        pr = np.concatenate(
            [w1[cls_of_p], b1[cls_of_p], w2[cls_of_p], b2[cls_of_p][:, None]],
            axis=1).astype(np.float32)
        xc = np.ascontiguousarray(np.concatenate(
            [x_sb.astype(np.float16), pr.view(np.float16)], axis=1))

        A = Wt[Fx[:, :NT].T]                            # [t, p, k, o] fp16
        A = A.transpose(0, 2, 1, 3).reshape(K * NT, P, O)   # [tau=t*K+k, p, o]
        im = {"xd": xc,
              "Wp": np.ascontiguousarray(Wt[Fx[:M, NT]].reshape(M, K * O))}
        tau = 0
        for ci, sz in enumerate(CHUNKS):
            im[f"Wm{ci}"] = np.ascontiguousarray(
                A[tau:tau + sz].transpose(1, 0, 2).reshape(P, sz * O))
            tau += sz
        assert tau == K * NT - 2
        im["Wh1"] = np.ascontiguousarray(
            A[tau:, :, 512:].transpose(1, 0, 2).reshape(P, 1024))
        im["Wh0"] = np.ascontiguousarray(
            A[tau:, :, :512].transpose(1, 0, 2).reshape(P, 1024))
        in_maps.append(im)

    allf = np.sort(np.concatenate(seen))
    assert allf.shape == (I,) and np.array_equal(allf, np.arange(I))

    nc = _build()
    res = run_bass_kernel_spmd(nc, in_maps, list(range(NCORES)), trace=TRACE)
    LAST_RESULT = res
    out = np.zeros((B, O), dtype=np.float32)
    for c in range(NCORES):
        out += res.results[c]["out"]
    return out


# revision 34
# speedup vs baseline: 1.1187x; 1.0089x over previous
"""KAN layer kernel for 8 Trainium2 NeuronCores.

Math (reference):
    basis[b,f] = sum_h silu(x[b,f]*w1[f%K,h] + b1[f%K,h]) * w2[f%K,h] + b2[f%K]
    out[b,o]   = sum_f basis[b,f] * Wsum[o,f],   Wsum = W.sum(-1)   # [O,F]

Sharding: features split 8 ways (each core holds ~2048 of the 16384
features and produces a partial out[64,1024]; host sums the partials).

Per-core device program (memory-bound on streaming its ~21 MB W slice):
  - W is cast to fp16 on the host (tolerance is 2e-2; fp16 keeps us ~5e-4)
    and laid out [tile, k, partition, o] so the k-reduction folds into the
    matmul contraction: no accum-DMA chains, no on-chip k-sum. The stream
    is 10 HWDGE DMAs with 12-16 KB per-partition lines: big enough that the
    SDMA engines run gap-free (small chunks pay per-dma_start ring bubbles),
    small enough not to trip the sticky ~20% all-engine clock throttle that
    32-44 KB descriptors trigger. All W tiles are SBUF-resident (fp16 fits)
    so the stream never stalls on compute. This is the chip-level HBM
    roofline: the 8 cores' slices are disjoint, so W is read exactly once.
  - Features are permuted so each SBUF partition only holds features of a
    single f%K residue class. The silu affine params are then per-partition
    constants valid across every tile, so the basis needs 64 wide ACT
    instructions + 64 wide DVE accumulates instead of 256+256 narrow ones
    (ACT costs (N+352)/1.2 ns per instr -- narrow = overhead-dominated).
  - The basis is computed in 4 column groups (tiles 3|4|5|5) and W streams
    t-major, so the PE starts contracting chunk 0 as soon as group 0 is
    done (~20 us) and the run stays DMA-stream-bound. (No PE warm-up
    matmuls: extra concurrent activity trips the clock throttle.)
  - Slot grid is 17 deep per partition: 16 full [128 x 64b] basis tiles plus
    one 20-partition spill tile absorbing the residue-class remainders
    (class sizes aren't multiples of 16).
  - 2 PSUM banks accumulate out[64,1024] across all 85 contraction tiles.
    The last 2 W tiles are split by output column half (Wh1 then Wh0), so
    ps1 closes early and its PSUM copy + out store overlap ps0's final
    matmuls; the post-stream tail is ~4 matmuls + one copy + one store.
"""
import numpy as np

B, I, O, K, H = 64, 16384, 1024, 5, 16
NCORES = 8
P = 128                   # SBUF partitions
NT = 16                   # full tiles (main slot grid depth)
T = NT + 1                # slots per partition incl. spill slot
M = 20                    # spill-tile partition count (max residue spill)
TB = T * B                # basis free dim: 17*64 = 1088
# Basis tile groups (start, ntiles): the PE can begin contracting group 0
# ~8 us after the first silu instead of waiting for the whole 17-tile chain.
GROUPS = [(0, 3), (3, 4), (7, 5), (12, 5)]
# W tiles stream t-major (tau = t*K + k) so early chunks only need half A.
# 16 KB per-partition lines: big enough for a gap-free SDMA stream, small
# enough to dodge the PE clock throttle seen with 32-44 KB descriptors.
# The last 2 tiles are split by output column half (Wh1/Wh0) so ps1's
# accumulation closes early and its copy+store overlap ps0's final matmuls.
CHUNKS = [8, 8, 8, 8, 8, 8, 8, 8, 8, 6]
PR = 3 * H + 1            # per-partition silu params: w1,b1,w2 + b2 (fp32)
XC = TB + 2 * PR          # const cols: fp16 x block + fp32 params as col pairs

TRACE = False             # test.py sets True to capture an NTFF profile
LAST_RESULT = None


def _plan_core(feats_by_class):
    """Assign one core's features to the (partition, slot) grid.

    Returns (cls_of_p[P], F17[P, T]) with F17 holding feature ids, -1 = pad.
    Every partition holds features of exactly one f%K class; spill slots
    (slot NT) only on partitions < M.
    """
    ks = [len(f) for f in feats_by_class]
    n = [-(-k // T) for k in ks]              # ceil(k/17) partitions minimum
    spare = P - sum(n)
    assert spare >= 0, (ks, n)
    for _ in range(spare):                     # kill the biggest spills first
        spills = [max(0, ks[c] - NT * n[c]) for c in range(K)]
        c = int(np.argmax(spills))
        n[c] += 1
    units = []                                 # (class, main[<=16], spill|-1)
    for c in range(K):
        fs = list(feats_by_class[c])
        main, sp = fs[: NT * n[c]], fs[NT * n[c]:]
        assert len(sp) <= n[c]
        for i in range(n[c]):
            units.append((c, main[NT * i: NT * (i + 1)],
                          sp[i] if i < len(sp) else -1))
    units.sort(key=lambda u: u[2] < 0)         # spill-carrying partitions first
    assert len(units) <= P
    n_spill = sum(1 for u in units if u[2] >= 0)
    assert n_spill <= M, n_spill
    units += [(0, [], -1)] * (P - len(units))
    cls_of_p = np.array([u[0] for u in units], dtype=np.int64)
    F17 = np.full((P, T), -1, dtype=np.int64)
    for p, (c, main, sp) in enumerate(units):
        F17[p, : len(main)] = main
        F17[p, NT] = sp
    return cls_of_p, F17


def _build():
    from contextlib import ExitStack
    from concourse import bacc, mybir, tile

    f32, f16 = mybir.dt.float32, mybir.dt.float16
    mult, add = mybir.AluOpType.mult, mybir.AluOpType.add
    nc = bacc.Bacc("TRN2", target_bir_lowering=False, debug=False,
                   num_devices=NCORES)
    Wms = [nc.declare_dram_parameter(f"Wm{i}", [P, sz * O], f16, isOutput=False)
           for i, sz in enumerate(CHUNKS)]
    Wh1 = nc.declare_dram_parameter("Wh1", [P, 1024], f16, isOutput=False)
    Wh0 = nc.declare_dram_parameter("Wh0", [P, 1024], f16, isOutput=False)
    Wp = nc.declare_dram_parameter("Wp", [M, K * O], f16, isOutput=False)
    xd = nc.declare_dram_parameter("xd", [P, XC], f16, isOutput=False)
    out = nc.declare_dram_parameter("out", [B, O], f32, isOutput=True)

    with tile.TileContext(nc) as tc, ExitStack() as ctx:
        const = ctx.enter_context(tc.tile_pool(name="const", bufs=1))
        wpool = ctx.enter_context(tc.tile_pool(name="w", bufs=1))
        wsp = ctx.enter_context(tc.tile_pool(name="wsp", bufs=1))
        spool = ctx.enter_context(tc.tile_pool(name="silu", bufs=4))
        apool = ctx.enter_context(tc.tile_pool(name="acc", bufs=1))
        opool = ctx.enter_context(tc.tile_pool(name="out", bufs=1))
        psum = ctx.enter_context(tc.tile_pool(name="psum", bufs=1, space="PSUM"))

        # x+params first on the SP HWDGE queue (basis can start ~3 us after
        # the preamble), then the ~21 MB W stream owns it end to end.
        xt = const.tile([P, XC], f16)
        nc.sync.dma_start(xt[:, :], xd[:, :])
        # Wp rides the same sync queue mid-stream (needed ~15 us after it
        # lands; avoids SWDGE entirely — its descriptor-ring traffic is the
        # suspected cause of the SDMA engine 7/15 slowdown).
        wtiles = []
        wpt = wsp.tile([M, K * O], f16)
        for c, sz in enumerate(CHUNKS):
            wt = wpool.tile([P, sz * O], f16, name=f"w{c}")
            nc.sync.dma_start(wt[:, :], Wms[c][:, :])
            wtiles.append((wt, sz))
            if c == 2:
                nc.sync.dma_start(wpt[:, :], Wp[:, :])
        wh1 = wpool.tile([P, 1024], f16, name="wh1")
        nc.sync.dma_start(wh1[:, :], Wh1[:, :])
        wh0 = wpool.tile([P, 1024], f16, name="wh0")
        nc.sync.dma_start(wh0[:, :], Wh0[:, :])

        # ---- basis: acc[p, t*B+b] = sum_h silu(x*w1+b1)*w2 + b2, fp16,
        # in 4 column groups so the PE can start on group 0 early ----
        accs = []

        def prm(i):          # i-th fp32 param, stored as an fp16 col pair
            return xt[:, TB + 2 * i:TB + 2 * i + 2].bitcast(f32)

        for gi, (t0, nt) in enumerate(GROUPS):
            lo, w = t0 * B, nt * B
            acc = apool.tile([P, w], f16, name=f"acc{gi}")
            accs.append(acc)
            for h in range(H):
                st = spool.tile([P, w], f16, tag=f"st{gi}")
                nc.scalar.activation(
                    st[:, :], xt[:, lo:lo + w],
                    mybir.ActivationFunctionType.Silu,
                    bias=prm(H + h), scale=prm(h))
                if h == 0:
                    nc.vector.tensor_scalar(
                        acc[:, :], st[:, :], prm(2 * H), prm(3 * H),
                        op0=mult, op1=add)
                else:
                    nc.vector.scalar_tensor_tensor(
                        acc[:, :], st[:, :], prm(2 * H + h),
                        acc[:, :], op0=mult, op1=add)

        # ---- matmuls: out[b,o] += acc_tile.T @ W_tile over 85 tiles ----
        ps0 = psum.tile([B, 512], f32, tag="ps0")
        ps1 = psum.tile([B, 512], f32, tag="ps1")
        nmm = K * NT + K          # accumulation length per PSUM bank
        n0 = n1 = 0

        def mm0(lhsT, rhs):
            nonlocal n0
            nc.tensor.matmul(ps0[:, :], lhsT, rhs,
                             start=(n0 == 0), stop=(n0 == nmm - 1))
            n0 += 1

        def mm1(lhsT, rhs):
            nonlocal n1
            nc.tensor.matmul(ps1[:, :], lhsT, rhs,
                             start=(n1 == 0), stop=(n1 == nmm - 1))
            n1 += 1

        def lhs_of(t):
            gi = max(i for i, (t0, _) in enumerate(GROUPS) if t >= t0)
            lo = (t - GROUPS[gi][0]) * B
            return accs[gi][:, lo:lo + B]

        tau = 0
        for ci, (wt, sz) in enumerate(wtiles):
            if ci == 7:   # spill (needs the last group) off the critical tail
                sp = accs[-1][0:M, (NT - GROUPS[-1][0]) * B:]
                for k in range(K):
                    mm0(sp, wpt[0:M, k * O:k * O + 512])
                    mm1(sp, wpt[0:M, k * O + 512:(k + 1) * O])
            for g in range(sz):
                lhsT = lhs_of(tau // K)
                mm0(lhsT, wt[:, g * O:g * O + 512])
                mm1(lhsT, wt[:, g * O + 512:(g + 1) * O])
                tau += 1
        # final 2 tiles, ps1 columns first: ps1 closes while ps0's last
        # matmuls still stream, so its copy + store overlap them.
        for g in range(2):
            mm1(lhs_of((tau + g) // K), wh1[:, g * 512:(g + 1) * 512])
        out_sb = opool.tile([B, O], f32)
        nc.scalar.copy(out_sb[:, 512:O], ps1[:, :])
        nc.sync.dma_start(out[:, 512:O], out_sb[:, 512:O])
        for g in range(2):
            mm0(lhs_of((tau + g) // K), wh0[:, g * 512:(g + 1) * 512])
        tau += 2
        assert tau == K * NT and n0 == nmm and n1 == nmm

        nc.vector.tensor_copy(out_sb[:, 0:512], ps0[:, :])
        nc.sync.dma_start(out[:, 0:512], out_sb[:, 0:512])
    nc.compile()
    return nc


def kernel(x, w1, b1, w2, b2, W):
    global LAST_RESULT
    from concourse.bass_utils import run_bass_kernel_spmd

    x = np.asarray(x, dtype=np.float32)
    W = np.asarray(W, dtype=np.float32)
    w1 = np.asarray(w1, dtype=np.float32)
    b1 = np.asarray(b1, dtype=np.float32)
    w2 = np.asarray(w2, dtype=np.float32)
    b2 = np.asarray(b2, dtype=np.float32)

    # ---- host prep: W -> fp16 [f, k, o] with a zero row for pad slots ----
    Wt = np.zeros((I + 1, K, O), dtype=np.float16)
    Wt[:I] = W.transpose(1, 2, 0)
    xp = np.concatenate([x, np.zeros((B, 1), np.float32)], axis=1)

    in_maps = []
    seen = []
    for j in range(NCORES):
        feats = [np.arange(c, I, K)[j::NCORES] for c in range(K)]
        cls_of_p, F17 = _plan_core(feats)
        seen.append(F17[F17 >= 0].ravel())

        Fx = np.where(F17 < 0, I, F17)                 # pad -> zero col/row
        x_sb = xp[:, Fx].transpose(1, 2, 0).reshape(P, TB)
        pr = np.concatenate(
            [w1[cls_of_p], b1[cls_of_p], w2[cls_of_p], b2[cls_of_p][:, None]],
            axis=1).astype(np.float32)
        xc = np.ascontiguousarray(np.concatenate(
            [x_sb.astype(np.float16), pr.view(np.float16)], axis=1))

        A = Wt[Fx[:, :NT].T]                            # [t, p, k, o] fp16
        A = A.transpose(0, 2, 1, 3).reshape(K * NT, P, O)   # [tau=t*K+k, p, o]
        im = {"xd": xc,
              "Wp": np.ascontiguousarray(Wt[Fx[:M, NT]].reshape(M, K * O))}
        tau = 0
        for ci, sz in enumerate(CHUNKS):
            im[f"Wm{ci}"] = np.ascontiguousarray(
                A[tau:tau + sz].transpose(1, 0, 2).reshape(P, sz * O))
            tau += sz
        assert tau == K * NT - 2
        im["Wh1"] = np.ascontiguousarray(
            A[tau:, :, 512:].transpose(1, 0, 2).reshape(P, 1024))
        im["Wh0"] = np.ascontiguousarray(
            A[tau:, :, :512].transpose(1, 0, 2).reshape(P, 1024))
        in_maps.append(im)

    allf = np.sort(np.concatenate(seen))
    assert allf.shape == (I,) and np.array_equal(allf, np.arange(I))

    nc = _build()
    res = run_bass_kernel_spmd(nc, in_maps, list(range(NCORES)), trace=TRACE)
    LAST_RESULT = res
    out = np.zeros((B, O), dtype=np.float32)
    for c in range(NCORES):
        out += res.results[c]["out"]
    return out
